# revision 1
# baseline (speedup 1.0000x reference)
"""TENER-style MultiHeadedAttention TRN2 kernel (8 NeuronCores, SPMD).

Sharding: core c handles batch b = c//4 and query rows [256*(c%4), 256*(c%4)+256).
Each core computes its full output slice o[b, s_slice, :]; host gather is pure
concatenation (no reduction).

Key math: the TENER relative-position term after the shift trick is
  rel[s, j] = (q_s + v_bias_h) . pos[S + j - s]
and pos rows are sinusoids, so by angle addition
  rel[s, j] = a_sin(s) . sin(w j) + a_cos(s) . cos(w j)
with a_sin = qv_sin*cos(w s) + qv_cos*sin(w s), a_cos = qv_cos*cos(w s) - qv_sin*sin(w s).
This turns (qk + rel) into ONE 128-deep contraction per head:
  scoresT[j, s] = [k_j ; sin(w j) ; cos(w j)] . [q_s ; a_sin(s) ; a_cos(s)]
eliminating the [S, 2S] intermediate and the diagonal shift entirely.

All matmuls run as float32r (fp32 with 11-bit mantissa, full PE rate).
Softmax denominators come free via a ones-column appended per head to v;
normalization uses a PE broadcast of the reciprocal row.
"""

import math
import sys

sys.path.insert(0, "/opt/trn_rl_repo")

import numpy as np

B, S, D = 2, 1024, 1024
H, HD = 16, 64          # heads, head_dim
HALF = 32               # sin/cos half of head_dim
NC_ = 8                 # cores
SP = 256                # query rows per core
JT = S // 128           # 8 key tiles
FT = D // 128           # 8 feature tiles

_cache: dict = {}


def _rne_fp32r(a):
    """Round fp32 -> fp32r (1s+8e+11m) with round-to-nearest-even."""
    u = np.ascontiguousarray(a, dtype=np.float32).view(np.uint32)
    lsb = (u >> np.uint32(12)) & np.uint32(1)
    return ((u + np.uint32(0x7FF) + lsb) & np.uint32(0xFFFFF000)).view(np.float32)


def _build_nc():
    import concourse.bacc as bacc
    import concourse.mybir as mybir
    from concourse import tile

    F32 = mybir.dt.float32
    F32R = mybir.dt.float32r
    ADD = mybir.AluOpType.add
    SUB = mybir.AluOpType.subtract
    MUL = mybir.AluOpType.mult
    DIV = mybir.AluOpType.divide
    EXP = mybir.ActivationFunctionType.Exp

    nc = bacc.Bacc("TRN2", target_bir_lowering=False, debug=False, num_devices=NC_)

    qpack = nc.dram_tensor("qpack", [D + 1, D + SP], F32R, kind="ExternalInput")
    wvpack = nc.dram_tensor("wvpack", [D, D], F32R, kind="ExternalInput")
    vtpack = nc.dram_tensor("vtpack", [D, D], F32R, kind="ExternalInput")
    kgd = nc.dram_tensor("kg", [2 * D, S], F32R, kind="ExternalInput")
    wopack = nc.dram_tensor("wopack", [D + 1, D], F32R, kind="ExternalInput")
    # tabs: [128, 256 CC | 256 SS | 64 ones | 8 vb-cols]
    tabs_d = nc.dram_tensor("tabs", [128, 840], F32R, kind="ExternalInput")
    out_d = nc.dram_tensor("out", [SP, D], F32, kind="ExternalOutput")

    with tile.TileContext(nc, num_cores=NC_) as tc:
        # ---------- persistent pools ----------
        with tc.tile_pool(name="persist", bufs=1) as pp, \
             tc.tile_pool(name="small", bufs=2) as sp, \
             tc.tile_pool(name="exppool", bufs=8) as ep:

            tabs = pp.tile([128, 840], F32R, tag="tabs")
            nc.sync.dma_start(tabs[:], tabs_d.ap())
            tabsf = tabs[:].bitcast(F32)

            kgt = []
            for tt in range(H // 2):
                t = pp.tile([128, 2 * S], F32R, name=f"kgt{tt}", tag=f"kgt{tt}")
                kgt.append(t)
            kg = [kgt[hh // 2][:, (hh % 2) * S:(hh % 2) * S + S] for hh in range(H)]

            catq = [pp.tile([128, SP], F32R, name=f"catq{hh}", tag=f"catq{hh}") for hh in range(H)]
            vv = [pp.tile([128, H * 65], F32R, name=f"vv{j}", tag=f"vv{j}") for j in range(JT)]
            xn = [pp.tile([128, SP], F32R, name=f"xn{c}", tag=f"xn{c}") for c in range(FT)]
            xn9 = tabs[0:1, 584:840]
            ebias = pp.tile([128, 1], F32, tag="ebias")
            nc.vector.memset(ebias[:], -25.0)


            # ---------- phase 1: q projection + rotation ----------
            with tc.tile_pool(name="qpk", bufs=1) as qpkp, \
                 tc.tile_pool(name="qps", bufs=2, space="PSUM") as qps:
                qpk = []
                for c in range(FT):
                    t = qpkp.tile([128, D + SP], F32R, name=f"qpk{c}", tag=f"qpk{c}")
                    eng = nc.sync if c % 2 == 0 else nc.gpsimd
                    eng.dma_start(t[:], qpack.ap()[c * 128:(c + 1) * 128, :])
                    qpk.append(t)
                qpk9 = qpkp.tile([1, D + SP], F32R, tag="qpk9", bufs=1)
                nc.sync.dma_start(qpk9[:], qpack.ap()[D:D + 1, :])

                for ft in range(FT):
                    qpsum = qps.tile([128, SP], F32, tag="qpsum")
                    for c in range(FT):
                        nc.tensor.matmul(
                            qpsum[:], qpk[c][:, ft * 128:(ft + 1) * 128],
                            qpk[c][:, D:D + SP], start=(c == 0), stop=False)
                    nc.tensor.matmul(qpsum[:], qpk9[:, ft * 128:(ft + 1) * 128],
                                     qpk9[:, D:D + SP], start=False, stop=True)

                    # q halves into catQ rows 0:64 (ACT partition-shift copies)
                    nc.scalar.copy(catq[2 * ft][0:64, :], qpsum[0:64, :])
                    nc.scalar.copy(catq[2 * ft + 1][0:64, :], qpsum[64:128, :])

                    # rotation -> catQ rows 64:128
                    qv = sp.tile([128, SP], F32, tag="qv")
                    nc.vector.tensor_scalar(
                        out=qv[:], in0=qpsum[:],
                        scalar1=tabsf[:, 576 + ft:577 + ft], scalar2=None, op0=ADD)
                    t1 = sp.tile([128, SP], F32, tag="t1")
                    nc.vector.tensor_tensor(out=t1[:], in0=qv[:],
                                            in1=tabsf[:, 0:SP], op=MUL)
                    t2 = sp.tile([128, SP], F32, tag="t2")
                    for g in range(4):
                        src = [32, 0, 96, 64][g]
                        nc.vector.tensor_tensor(
                            out=t2[g * 32:(g + 1) * 32, :],
                            in0=qv[src:src + 32, :],
                            in1=tabsf[src:src + 32, SP:2 * SP], op=MUL)
                    for par in range(2):
                        hq = 2 * ft + par
                        o_ = par * 64
                        nc.vector.tensor_tensor(
                            out=catq[hq][64:96, :], in0=t1[o_:o_ + 32, :],
                            in1=t2[o_:o_ + 32, :], op=ADD)
                        nc.vector.tensor_tensor(
                            out=catq[hq][96:128, :], in0=t1[o_ + 32:o_ + 64, :],
                            in1=t2[o_ + 32:o_ + 64, :], op=SUB)

            # ---------- phase 2: v projection (runs after qpack DMAs; kg later) ----------
            with tc.tile_pool(name="wvp", bufs=1) as wvpp, \
                 tc.tile_pool(name="valp", bufs=2) as valpp, \
                 tc.tile_pool(name="vps", bufs=3, space="PSUM") as vps:
                wvp = []
                for c in range(FT):
                    t = wvpp.tile([128, D], F32R, name=f"wvp{c}", tag=f"wvp{c}")
                    wvp.append(t)


                valts = {}
                for jh in range(2):
                    valts[jh] = []
                    for c in range(FT):
                        t = valpp.tile([128, 512], F32R, name=f"val{c}_{jh}", tag=f"val{c}")
                        valts[jh].append(t)

                def val_dmas(jh):
                    for c in range(FT):
                        nc.gpsimd.dma_start(
                            valts[jh][c][:], vtpack.ap()[c * 128:(c + 1) * 128,
                                                         jh * 512:(jh + 1) * 512])

                def vproj_half(jh):
                    valt = valts[jh]
                    for jq in range(4):
                        jt = jh * 4 + jq
                        vvt = vv[jt]
                        nc.scalar.copy(
                            vvt[:].rearrange("p (h x) -> p h x", x=65)[:, :, 64:65],
                            tabs[:, 512:528].rearrange("p (h x) -> p h x", x=1))
                        for hf in range(2):
                            vpsum = vps.tile([128, 512], F32, tag="vpsum")
                            for c in range(FT):
                                nc.tensor.matmul(
                                    vpsum[:],
                                    valt[c][:, jq * 128:(jq + 1) * 128],
                                    wvp[c][:, hf * 512:(hf + 1) * 512],
                                    start=(c == 0), stop=(c == FT - 1))
                            dst = vvt[:, hf * 520:(hf + 1) * 520].rearrange(
                                "p (h x) -> p h x", x=65)[:, :, 0:64]
                            src_ = vpsum[:].rearrange("p (h d) -> p h d", d=64)
                            nc.scalar.copy(dst, src_)

                val_dmas(0)
                for c in range(FT):
                    nc.sync.dma_start(wvp[c][:], wvpack.ap()[c * 128:(c + 1) * 128, :])
                val_dmas(1)
                vproj_half(0)
                for tt in range(H // 2):
                    eng = nc.sync if tt % 2 == 0 else nc.gpsimd
                    eng.dma_start(
                        kgt[tt][:].rearrange("p (a s) -> p a s", a=2),
                        kgd.ap()[tt * 256:(tt + 1) * 256, :].rearrange(
                            "(a p) s -> p a s", p=128))
                vproj_half(1)

            # ---------- phase 3: attention ----------
            with tc.tile_pool(name="wop", bufs=1) as wop:
                wo = []
                for c in range(FT):
                    t = wop.tile([128, D], F32R, tag=f"wo{c}")
                    nc.gpsimd.dma_start(t[:], wopack.ap()[c * 128:(c + 1) * 128, :])
                    wo.append(t)
                wo9 = wop.tile([1, D], F32R, tag="wo9")
                nc.sync.dma_start(wo9[:], wopack.ap()[D:D + 1, :])

                with tc.tile_pool(name="scps", bufs=4, space="PSUM") as scps, \
                     tc.tile_pool(name="xtps", bufs=2, space="PSUM") as xtps, \
                     tc.tile_pool(name="rbps", bufs=2, space="PSUM") as rbps:
                  for hh in range(H):
                    xt = xtps.tile([65, SP], F32, tag="xt")
                    for jt in range(JT):
                        sc = scps.tile([128, SP], F32, tag="sc")
                        nc.tensor.matmul(
                            sc[:], kgt[hh // 2][:, (hh % 2) * S + jt * 128:
                                               (hh % 2) * S + (jt + 1) * 128],
                            catq[hh][:], start=True, stop=True,
                            skip_group_check=True)
                        ex = ep.tile([128, SP], F32R, tag="ex")
                        nc.scalar.activation(ex[:], sc[:], EXP, bias=ebias[:], scale=1.0)
                        nc.tensor.matmul(
                            xt[0:65, :], vv[jt][:, hh * 65:hh * 65 + 65], ex[:],
                            start=(jt == 0), stop=(jt == JT - 1),
                            skip_group_check=True)
                    # normalize: bcast denom row then divide
                    drow = sp.tile([128, SP], F32R, tag="drow")
                    nc.scalar.copy(drow[64:65, :], xt[64:65, :])
                    rb = rbps.tile([64, SP], F32, tag="rb")
                    nc.tensor.matmul(rb[:], tabs[64:65, 512:576], drow[64:65, :],
                                     start=True, stop=True, skip_group_check=True)
                    rrec = sp.tile([64, SP], F32, tag="rrec")
                    nc.vector.reciprocal(rrec[:], rb[0:64, :])
                    nc.vector.tensor_tensor(
                        out=xn[hh // 2][(hh % 2) * 64:(hh % 2) * 64 + 64, :],
                        in0=xt[0:64, :], in1=rrec[:], op=MUL)

                # ---------- phase 4: output projection ----------
                with tc.tile_pool(name="ops", bufs=2, space="PSUM") as ops, \
                     tc.tile_pool(name="osb", bufs=2) as osb:
                    for st in range(2):
                        for hf in range(2):
                            op = ops.tile([128, 512], F32, tag="op")
                            for c in range(FT):
                                nc.tensor.matmul(
                                    op[:], xn[c][:, st * 128:(st + 1) * 128],
                                    wo[c][:, hf * 512:(hf + 1) * 512],
                                    start=(c == 0), stop=False)
                            nc.tensor.matmul(
                                op[:], xn9[:, st * 128:(st + 1) * 128],
                                wo9[:, hf * 512:(hf + 1) * 512],
                                start=False, stop=True)
                            os_ = osb.tile([128, 512], F32, tag="os")
                            nc.scalar.copy(os_[:], op[:])
                            nc.sync.dma_start(
                                out_d.ap()[st * 128:(st + 1) * 128,
                                           hf * 512:(hf + 1) * 512], os_[:])

    nc.finalize()
    return nc


def _host_pack(query, key, value, Wq, bq, Wv, bv, Wo, bo, v_bias):
    """Build the 8 per-core input maps."""
    r = _rne_fp32r
    w = np.exp(np.arange(HALF) * (-math.log(10000.0) / (HALF - 1))).astype(np.float64)

    WqT = np.concatenate([Wq.T, bq[None, :]], axis=0)          # [1025, 1024]
    bo_eff = bo + Wo @ bv                                      # bv folds out via softmax sum=1
    WoTb = np.concatenate([Wo.T, bo_eff[None, :]], axis=0)     # [1025, 1024]
    wopack = r(WoTb)
    wvpack_r = r(Wv.T)

    # g table [64, S]
    j = np.arange(S, dtype=np.float64)
    gsin = np.sin(w[:, None] * j[None, :])
    gcos = np.cos(w[:, None] * j[None, :])
    g64 = np.concatenate([gsin, gcos], axis=0).astype(np.float32)  # [64, S]

    kgs = []
    vpacks = []
    wvpack = None
    for b in range(B):
        kT = key[b].T  # [1024, 1024] (h,dh)-major rows
        kgb = np.empty((2 * D, S), np.float32)
        for hh in range(H):
            kgb[hh * 128:hh * 128 + 64] = kT[hh * 64:(hh + 1) * 64]
            kgb[hh * 128 + 64:hh * 128 + 128] = g64
        kgs.append(r(kgb))
        vpacks.append(r(value[b].T))

    vbflat = v_bias.reshape(-1).astype(np.float32)             # [1024] (h,dh)

    in_maps = []
    for c in range(NC_):
        b, sl = c // 4, c % 4
        s0 = sl * SP
        qp = np.empty((D + 1, D + SP), np.float32)
        qp[:D, :D] = WqT[:D]
        qp[:D, D:] = query[b].T[:, s0:s0 + SP]
        qp[D, :D] = WqT[D]
        qp[D, D:] = 1.0

        svals = (s0 + np.arange(SP, dtype=np.float64))[None, :]  # [1, 256]
        wrep = np.tile(w, 4)[:, None]                            # [128, 1]
        tabs = np.empty((128, 840), np.float32)
        tabs[:, 0:SP] = np.cos(wrep * svals)
        tabs[:, SP:2 * SP] = np.sin(wrep * svals)
        tabs[:, 512:576] = 1.0
        tabs[:, 576:584] = vbflat.reshape(8, 128).T
        tabs[:, 584:840] = 1.0

        in_maps.append({
            "qpack": r(qp),
            "wvpack": wvpack_r,
            "vtpack": vpacks[b],
            "kg": kgs[b],
            "wopack": wopack,
            "tabs": r(tabs),
        })
    return in_maps


def kernel(query, key, value, mask, Wq, bq, Wv, bv, Wo, bo, v_bias):
    from concourse.bass_utils import run_bass_kernel_spmd

    query = np.asarray(query, np.float32)
    key = np.asarray(key, np.float32)
    value = np.asarray(value, np.float32)
    in_maps = _host_pack(query, key, value,
                         np.asarray(Wq, np.float32), np.asarray(bq, np.float32),
                         np.asarray(Wv, np.float32), np.asarray(bv, np.float32),
                         np.asarray(Wo, np.float32), np.asarray(bo, np.float32),
                         np.asarray(v_bias, np.float32))

    if "nc" not in _cache:
        _cache["nc"] = _build_nc()
    nc = _cache["nc"]

    import os
    if int(os.environ.get("BASS_KERNEL_TRACE", "0")):
        import importlib.util as _ilu
        if "antenv.axon_hooks" not in sys.modules:
            _spec = _ilu.spec_from_file_location(
                "antenv.axon_hooks", "/opt/trn_rl_repo/antenv/axon_hooks.py")
            _mod = _ilu.module_from_spec(_spec)
            _spec.loader.exec_module(_mod)
            sys.modules["antenv.axon_hooks"] = _mod
    res = run_bass_kernel_spmd(
        nc, in_maps, core_ids=list(range(NC_)),
        trace=bool(int(os.environ.get("BASS_KERNEL_TRACE", "0"))))
    _cache["last_result"] = res

    out = np.empty((B, S, D), np.float32)
    for c in range(NC_):
        b, sl = c // 4, c % 4
        out[b, sl * SP:(sl + 1) * SP, :] = res.results[c]["out"]
    return out



# revision 14
# speedup vs baseline: 1.4489x; 1.4489x over previous
"""TENER-style MultiHeadedAttention TRN2 kernel (8 NeuronCores, SPMD).

Sharding (tensor-parallel over heads x data-parallel over batch):
core c handles batch b = c//4 and the 4 heads [4*(c%4), 4*(c%4)+4),
over ALL 1024 query rows. Wq/Wv are split column-wise by head, Wo
row-wise; each core emits a PARTIAL output [S, D] and the host gather
sums the 4 partials per batch (the Wo all-reduce).

Key math: the TENER relative-position term after the shift trick is
  rel[s, j] = (q_s + v_bias_h) . pos[j - s]
and pos rows are sinusoids, so by angle addition the whole score is ONE
128-deep contraction per head:
  scores[j, s] = [k_j ; sin(w j) ; cos(w j)] . [q_s ; a_sin(s) ; a_cos(s)]
  a_sin = qv_sin*cos(w s) + qv_cos*sin(w s)
  a_cos = qv_cos*cos(w s) - qv_sin*sin(w s)

All q/k-path matmuls run as float32r (full PE rate at free-dim >= 256).
The v projection runs in bf16 (halves its DMA traffic; v errors are not
exp-amplified). Softmax denominators come free via a ones-column per
head appended to v; normalization broadcasts the denominator row with a
rank-1 PE matmul and divides on DVE.
"""

import math
import os
import sys

sys.path.insert(0, "/opt/trn_rl_repo")

import numpy as np

B, S, D = 2, 1024, 1024
H, HD = 16, 64          # global heads, head_dim
HL = 4                  # local heads per core
HALF = 32               # sin/cos half of head_dim
NC_ = 8                 # cores
JT = S // 128           # 8 key tiles
CT = D // 128           # 8 contraction tiles

_cache: dict = {}


def _rne_fp32r(a):
    """Round fp32 -> fp32r (1s+8e+11m) with round-to-nearest-even."""
    u = np.ascontiguousarray(a, dtype=np.float32).view(np.uint32)
    lsb = (u >> np.uint32(12)) & np.uint32(1)
    return ((u + np.uint32(0x7FF) + lsb) & np.uint32(0xFFFFF000)).view(np.float32)


def _build_nc(has_bq: bool, has_bo: bool):
    import concourse.bacc as bacc
    import concourse.mybir as mybir
    from concourse import tile

    F32 = mybir.dt.float32
    F32R = mybir.dt.float32r
    BF16 = mybir.dt.bfloat16
    ADD = mybir.AluOpType.add
    MUL = mybir.AluOpType.mult
    DIV = mybir.AluOpType.divide
    EXP = mybir.ActivationFunctionType.Exp

    nc = bacc.Bacc("TRN2", target_bir_lowering=False, debug=False, num_devices=NC_)

    QW = HL * HD                      # 256 local q / v feature cols
    qrows = D + 1 if has_bq else D
    worows = QW + 1 if has_bo else QW
    qpack = nc.dram_tensor("qpack", [qrows, QW + S], F32R, kind="ExternalInput")
    wvp_d = nc.dram_tensor("wvp", [D, QW], BF16, kind="ExternalInput")
    vt_d = nc.dram_tensor("vt", [D, S], BF16, kind="ExternalInput")
    kg_d = nc.dram_tensor("kg", [HL * 128, S], F32R, kind="ExternalInput")
    wo_d = nc.dram_tensor("wo", [worows, D], F32R, kind="ExternalInput")
    # tabs: [128, 1024 coss | 1024 sinsw | 2 vb-cols]
    tabs_d = nc.dram_tensor("tabs", [128, 2 * S + 2], F32, kind="ExternalInput")
    out_d = nc.dram_tensor("out", [S, D], F32, kind="ExternalOutput")

    with tile.TileContext(nc, num_cores=NC_) as tc:
        with tc.tile_pool(name="persist", bufs=1) as pp, \
             tc.tile_pool(name="scratch", bufs=2) as sp, \
             tc.tile_pool(name="exppool", bufs=3) as ep:

            # ---------- persistent SBUF ----------
            tabs = pp.tile([128, 2 * S + 2], F32, tag="tabs")
            qall = pp.tile([128, CT * (QW + S)], F32R, tag="qall")
            kgt = pp.tile([128, HL * S], F32R, tag="kgt")
            wvall = pp.tile([128, CT * QW], BF16, tag="wvall")
            vtall = pp.tile([128, CT * S], BF16, tag="vtall")
            wot = pp.tile([128, 2 * D], F32R, tag="wot")
            catq = [pp.tile([128, S], F32R, name=f"catq{h}", tag=f"catq{h}")
                    for h in range(HL)]
            vv = [pp.tile([128, HL * (HD + 1)], F32R, name=f"vv{j}", tag=f"vv{j}")
                  for j in range(JT)]
            xn = [pp.tile([128, S], F32R, name=f"xn{c}", tag=f"xn{c}")
                  for c in range(2)]
            ebias = pp.tile([128, 1], F32, tag="ebias")
            if has_bq:
                qpk9 = pp.tile([1, QW + S], F32R, tag="qpk9")
            if has_bo:
                wo9 = pp.tile([1, D], F32R, tag="wo9")
                xn1 = pp.tile([1, 128], F32R, tag="xn1")

            # ---------- input DMAs (one per logical tensor) ----------
            nc.sync.dma_start(
                qall[:].rearrange("p (c n) -> p c n", c=CT),
                qpack.ap()[0:D, :].rearrange("(c p) n -> p c n", p=128))
            nc.gpsimd.dma_start(tabs[:], tabs_d.ap())
            nc.gpsimd.dma_start(
                wvall[:].rearrange("p (c n) -> p c n", c=CT),
                wvp_d.ap().rearrange("(c p) n -> p c n", p=128))
            nc.gpsimd.dma_start(
                vtall[:].rearrange("p (c n) -> p c n", c=CT),
                vt_d.ap().rearrange("(c p) n -> p c n", p=128))
            nc.sync.dma_start(
                kgt[:].rearrange("p (h n) -> p h n", h=HL),
                kg_d.ap().rearrange("(h p) n -> p h n", p=128))
            nc.gpsimd.dma_start(
                wot[:].rearrange("p (c n) -> p c n", c=2),
                wo_d.ap()[0:QW, :].rearrange("(c p) n -> p c n", p=128))
            if has_bq:
                nc.sync.dma_start(qpk9[:], qpack.ap()[D:D + 1, :])
            if has_bo:
                nc.sync.dma_start(wo9[:], wo_d.ap()[QW:QW + 1, :])

            # views
            qpk = [qall[:, c * (QW + S):(c + 1) * (QW + S)] for c in range(CT)]
            kg = [kgt[:, h * S:(h + 1) * S] for h in range(HL)]
            wv = [wvall[:, c * QW:(c + 1) * QW] for c in range(CT)]
            vt = [vtall[:, c * S:(c + 1) * S] for c in range(CT)]
            wo = [wot[:, c * D:(c + 1) * D] for c in range(2)]
            coss = tabs[:, 0:S]
            sinsw = tabs[:, S:2 * S]

            # ---------- small inits ----------
            nc.vector.memset(ebias[:], -25.0)
            if has_bo:
                nc.vector.memset(xn1[:], 1.0)
            for j in range(JT):
                for h in range(HL):
                    nc.vector.memset(
                        vv[j][:, h * (HD + 1) + HD:h * (HD + 1) + HD + 1]
                        .bitcast(F32), 1.0)

            # ---------- phase 1: q projection + rotation ----------
            with tc.tile_pool(name="qps", bufs=2, space="PSUM") as qps, \
                 tc.tile_pool(name="vps", bufs=2, space="PSUM") as vps:
                for ft in range(2):
                    qp = qps.tile([128, S], F32, tag="qp")
                    for half in range(2):
                        hs = slice(half * 512, half * 512 + 512)
                        for c in range(CT):
                            nc.tensor.matmul(
                                qp[:, hs],
                                qpk[c][:, ft * 128:(ft + 1) * 128],
                                qpk[c][:, QW + half * 512:QW + half * 512 + 512],
                                start=(c == 0),
                                stop=(c == CT - 1 and not has_bq),
                                skip_group_check=True)
                        if has_bq:
                            nc.tensor.matmul(
                                qp[:, hs], qpk9[:, ft * 128:(ft + 1) * 128],
                                qpk9[:, QW + half * 512:QW + half * 512 + 512],
                                start=False, stop=True, skip_group_check=True)

                    # q rows -> catq[0:64] (ACT partition-shift copies)
                    nc.scalar.copy(catq[2 * ft][0:64, :], qp[0:64, :])
                    nc.scalar.copy(catq[2 * ft + 1][0:64, :], qp[64:128, :])

                    # rotation -> catq rows 64:128
                    vbc = tabs[:, 2 * S + ft:2 * S + ft + 1]
                    t1 = sp.tile([128, S], F32, tag="t1")
                    nc.vector.scalar_tensor_tensor(
                        out=t1[:], in0=qp[:], scalar=vbc, in1=coss,
                        op0=ADD, op1=MUL)
                    # v2s[p] = (qp[swap32(p)] + vb[swap32(p)]) * sinsw[swap32(p)]
                    # (32-block swap folded into the shifted output base; the
                    # sign of the sin factor is folded into the sinsw table)
                    v2s = sp.tile([128, S], F32, tag="v2s")
                    for blk in range(4):
                        si = 32 * (blk ^ 1)
                        nc.vector.scalar_tensor_tensor(
                            out=v2s[32 * blk:32 * blk + 32, :],
                            in0=qp[si:si + 32, :],
                            scalar=tabs[si:si + 32, 2 * S + ft:2 * S + ft + 1],
                            in1=sinsw[si:si + 32, :],
                            op0=ADD, op1=MUL)
                    for par in range(2):
                        hq = 2 * ft + par
                        o_ = par * 64
                        eng = nc.vector if par == 0 else nc.gpsimd
                        eng.tensor_tensor(
                            out=catq[hq][64:128, :], in0=t1[o_:o_ + 64, :],
                            in1=v2s[o_:o_ + 64, :], op=ADD)

                # ---------- phase 2: v projection (bf16) ----------
                for jt in range(JT):
                    vp = vps.tile([128, QW], F32, tag="vp")
                    for c in range(CT):
                        nc.tensor.matmul(
                            vp[:], vt[c][:, jt * 128:(jt + 1) * 128], wv[c][:],
                            start=(c == 0), stop=(c == CT - 1),
                            skip_group_check=True)
                    dst = vv[jt][:].rearrange(
                        "p (h x) -> p h x", x=HD + 1)[:, :, 0:HD]
                    src_ = vp[:].rearrange("p (h d) -> p h d", d=HD)
                    nc.vector.tensor_copy(dst, src_)

            # ---------- phase 3: attention ----------
            with tc.tile_pool(name="scps", bufs=2, space="PSUM") as scps, \
                 tc.tile_pool(name="xtps", bufs=2, space="PSUM") as xtps:
                for h in range(HL):
                    xt = xtps.tile([128, S], F32, tag="xt")
                    for jt in range(JT):
                        sc = scps.tile([128, S], F32, tag="sc")
                        for half in range(2):
                            hs = slice(half * 512, half * 512 + 512)
                            nc.tensor.matmul(
                                sc[:, hs],
                                kg[h][:, jt * 128:(jt + 1) * 128],
                                catq[h][:, hs],
                                start=True, stop=True, skip_group_check=True)
                        ex = ep.tile([128, S], F32R, tag="ex")
                        nc.scalar.activation(ex[:], sc[:], EXP,
                                             bias=ebias[:], scale=1.0)
                        for half in range(2):
                            hs = slice(half * 512, half * 512 + 512)
                            nc.tensor.matmul(
                                xt[0:HD + 1, hs],
                                vv[jt][:, h * (HD + 1):(h + 1) * (HD + 1)],
                                ex[:, hs],
                                start=(jt == 0), stop=(jt == JT - 1),
                                skip_group_check=True)
                    # normalize: reciprocal of the denom row, partition-
                    # broadcast it into SBUF, multiply (xt is the only PSUM
                    # operand - verifier allows at most one).
                    dsb = sp.tile([1, S], F32, tag="dsb")
                    nc.vector.tensor_copy(dsb[0:1, :], xt[HD:HD + 1, :])
                    rsb = sp.tile([1, S], F32, tag="rsb")
                    nc.vector.reciprocal_approx_fast(out=rsb[0:1, :],
                                                     in_=dsb[0:1, :])
                    rbs = sp.tile([64, S], F32, tag="rbs")
                    nc.gpsimd.partition_broadcast(rbs[:], rsb[0:1, :])
                    nc.vector.tensor_tensor(
                        out=xn[h // 2][(h % 2) * 64:(h % 2) * 64 + 64, :],
                        in0=xt[0:HD, :], in1=rbs[:], op=MUL)

            # ---------- phase 4: output projection (partial out) ----------
            with tc.tile_pool(name="ops", bufs=2, space="PSUM") as ops, \
                 tc.tile_pool(name="osb", bufs=2) as osb:
                for qt in range(8):
                    op = ops.tile([128, D], F32, tag="op")
                    for half in range(2):
                        hs = slice(half * 512, half * 512 + 512)
                        for c in range(2):
                            nc.tensor.matmul(
                                op[:, hs], xn[c][:, qt * 128:(qt + 1) * 128],
                                wo[c][:, hs],
                                start=(c == 0), stop=(c == 1 and not has_bo),
                                skip_group_check=True)
                        if has_bo:
                            nc.tensor.matmul(
                                op[:, hs], xn1[:], wo9[:, hs],
                                start=False, stop=True, skip_group_check=True)
                    os_ = osb.tile([128, D], F32, tag="os")
                    if qt % 2 == 0:
                        nc.scalar.copy(os_[:], op[:])
                    else:
                        nc.vector.tensor_copy(os_[:], op[:])
                    deng = nc.sync if qt % 2 == 0 else nc.gpsimd
                    deng.dma_start(out_d.ap()[qt * 128:(qt + 1) * 128, :], os_[:])

    nc.finalize()
    return nc


def _host_pack(query, key, value, Wq, bq, Wv, bv, Wo, bo, v_bias):
    """Build the 8 per-core input maps."""
    import ml_dtypes
    r = _rne_fp32r
    bf = ml_dtypes.bfloat16
    QW = HL * HD
    w = np.exp(np.arange(HALF) * (-math.log(10000.0) / (HALF - 1)))

    has_bq = bool(np.any(bq))
    has_bo = bool(np.any(bo)) or bool(np.any(bv))

    # tables shared across the 4 cores of a batch except vb cols
    j = np.arange(S, dtype=np.float64)
    ang_j = w[:, None] * j[None, :]                      # [32, S]
    g64 = np.concatenate([np.sin(ang_j), np.cos(ang_j)], axis=0).astype(np.float32)

    wrep = np.tile(w, 4)[:, None]                        # [128, 1]
    svals = np.arange(S, dtype=np.float64)[None, :]
    cos_ws = np.cos(wrep * svals).astype(np.float32)     # [128, S]
    sin_ws = np.sin(wrep * svals).astype(np.float32)
    sinsw = sin_ws.copy()
    sinsw[0:32] *= -1.0
    sinsw[64:96] *= -1.0

    WqT = Wq.T.astype(np.float32)                        # [D, D]
    WvT = Wv.T.astype(np.float32)
    WoT = Wo.T.astype(np.float32)                        # [Dv, D]

    qTs, kTs, vTs = [], [], []
    for b in range(B):
        qTs.append(np.ascontiguousarray(query[b].T))
        kTs.append(np.ascontiguousarray(key[b].T))
        vTs.append(value[b].T.astype(bf))

    in_maps = []
    for c in range(NC_):
        b, g = c // 4, c % 4
        col0 = g * QW

        qp = np.empty((D + 1 if has_bq else D, QW + S), np.float32)
        qp[:D, :QW] = WqT[:, col0:col0 + QW]
        qp[:D, QW:] = qTs[b]
        if has_bq:
            qp[D, :QW] = bq[col0:col0 + QW]
            qp[D, QW:] = 1.0

        kg = np.empty((HL * 128, S), np.float32)
        for h in range(HL):
            gh = 4 * g + h
            kg[h * 128:h * 128 + 64] = kTs[b][gh * 64:(gh + 1) * 64]
            kg[h * 128 + 64:h * 128 + 128] = g64

        worows = QW + 1 if has_bo else QW
        wop = np.empty((worows, D), np.float32)
        wop[:QW] = WoT[col0:col0 + QW, :]
        if has_bo:
            wop[QW] = bo / 4.0 + bv[col0:col0 + QW] @ WoT[col0:col0 + QW, :]

        tabs = np.empty((128, 2 * S + 2), np.float32)
        tabs[:, 0:S] = cos_ws
        tabs[:, S:2 * S] = sinsw
        vbflat = v_bias.reshape(-1).astype(np.float32)
        for ft in range(2):
            tabs[:, 2 * S + ft] = vbflat[(4 * g + 2 * ft) * 64:
                                         (4 * g + 2 * ft + 2) * 64]

        in_maps.append({
            "qpack": r(qp),
            "wvp": WvT[:, col0:col0 + QW].astype(bf),
            "vt": vTs[b],
            "kg": r(kg),
            "wo": r(wop),
            "tabs": tabs,
        })
    return in_maps, has_bq, has_bo


def kernel(query, key, value, mask, Wq, bq, Wv, bv, Wo, bo, v_bias):
    from concourse.bass_utils import run_bass_kernel_spmd

    query = np.asarray(query, np.float32)
    key = np.asarray(key, np.float32)
    value = np.asarray(value, np.float32)
    in_maps, has_bq, has_bo = _host_pack(
        query, key, value,
        np.asarray(Wq, np.float32), np.asarray(bq, np.float32),
        np.asarray(Wv, np.float32), np.asarray(bv, np.float32),
        np.asarray(Wo, np.float32), np.asarray(bo, np.float32),
        np.asarray(v_bias, np.float32))

    ckey = ("nc", has_bq, has_bo)
    if ckey not in _cache:
        _cache[ckey] = _build_nc(has_bq, has_bo)
    nc = _cache[ckey]

    res = run_bass_kernel_spmd(
        nc, in_maps, core_ids=list(range(NC_)),
        trace=bool(int(os.environ.get("BASS_KERNEL_TRACE", "0"))))
    _cache["last_result"] = res

    out = np.empty((B, S, D), np.float32)
    for b in range(B):
        acc = res.results[4 * b]["out"].astype(np.float32)
        for g in range(1, 4):
            acc = acc + res.results[4 * b + g]["out"]
        out[b] = acc
    return out


# revision 23
# speedup vs baseline: 1.5849x; 1.0939x over previous
"""TENER-style MultiHeadedAttention TRN2 kernel (8 NeuronCores, SPMD).

Sharding (tensor-parallel over heads x data-parallel over batch):
core c handles batch b = c//4 and the 4 heads [4*(c%4), 4*(c%4)+4),
over ALL 1024 query rows. Wq/Wv are split column-wise by head, Wo
row-wise; each core emits a PARTIAL output [S, D] and the host gather
sums the 4 partials per batch (the Wo all-reduce).

Key math: the TENER relative-position term after the shift trick is
  rel[s, j] = (q_s + v_bias_h) . pos[j - s]
and pos rows are sinusoids, so by angle addition the whole score is ONE
128-deep contraction per head:
  scores[j, s] = [k_j ; sin(w j) ; cos(w j)] . [q_s ; a_sin(s) ; a_cos(s)]
  a_sin = qv_sin*cos(w s) + qv_cos*sin(w s)
  a_cos = qv_cos*cos(w s) - qv_sin*sin(w s)

All q/k-path matmuls run as float32r (full PE rate at free-dim >= 256).
The v projection runs in bf16 (halves its DMA traffic; v errors are not
exp-amplified). Softmax denominators come free via a ones-column per
head appended to v; normalization broadcasts the denominator row with a
rank-1 PE matmul and divides on DVE.
"""

import math
import os
import sys

sys.path.insert(0, "/opt/trn_rl_repo")

import numpy as np

B, S, D = 2, 1024, 1024
H, HD = 16, 64          # global heads, head_dim
HL = 4                  # local heads per core
HALF = 32               # sin/cos half of head_dim
NC_ = 8                 # cores
JT = S // 128           # 8 key tiles
CT = D // 128           # 8 contraction tiles

_cache: dict = {}


def _rne_fp32r(a):
    """Round fp32 -> fp32r (1s+8e+11m) with round-to-nearest-even."""
    u = np.ascontiguousarray(a, dtype=np.float32).view(np.uint32)
    lsb = (u >> np.uint32(12)) & np.uint32(1)
    return ((u + np.uint32(0x7FF) + lsb) & np.uint32(0xFFFFF000)).view(np.float32)


def _build_nc(has_bq: bool, has_bo: bool):
    import concourse.bacc as bacc
    import concourse.mybir as mybir
    from concourse import tile

    F32 = mybir.dt.float32
    F32R = mybir.dt.float32r
    BF16 = mybir.dt.bfloat16
    ADD = mybir.AluOpType.add
    MUL = mybir.AluOpType.mult
    DIV = mybir.AluOpType.divide
    EXP = mybir.ActivationFunctionType.Exp

    nc = bacc.Bacc("TRN2", target_bir_lowering=False, debug=False, num_devices=NC_)

    QW = HL * HD                      # 256 local q / v feature cols
    worows = QW + 1 if has_bo else QW
    # qpack: [Wq_loc.T (256) | Wq_loc_swapped.T (256) | query.T (S)], plus a
    # bias row: [bq_loc | bq_sw + vb_sw | ones] (the swapped projection feeds
    # the rotation and always carries the v_bias).
    qpack = nc.dram_tensor("qpack", [D + 1, 2 * QW + S], F32R,
                           kind="ExternalInput")
    wvp_d = nc.dram_tensor("wvp", [D, QW], BF16, kind="ExternalInput")
    vt_d = nc.dram_tensor("vt", [D, S], BF16, kind="ExternalInput")
    kg_d = nc.dram_tensor("kg", [HL * 128, S], F32R, kind="ExternalInput")
    wo_d = nc.dram_tensor("wo", [worows, D], F32R, kind="ExternalInput")
    # tabs: [128, 1024 coss | 1024 sinsw | 2 vb-cols]
    tabs_d = nc.dram_tensor("tabs", [128, 2 * S + 2], F32, kind="ExternalInput")
    out_d = nc.dram_tensor("out", [S, D], F32, kind="ExternalOutput")

    with tile.TileContext(nc, num_cores=NC_) as tc:
        with tc.tile_pool(name="persist", bufs=1) as pp, \
             tc.tile_pool(name="scratch", bufs=2) as sp, \
             tc.tile_pool(name="exppool", bufs=3) as ep:

            # ---------- persistent SBUF ----------
            tabs = pp.tile([128, 2 * S + 2], F32, tag="tabs")
            qall = pp.tile([128, CT * (2 * QW + S)], F32R, tag="qall")
            kgt = pp.tile([128, HL * S], F32R, tag="kgt")
            wvall = pp.tile([128, CT * QW], BF16, tag="wvall")
            vtall = pp.tile([128, CT * S], BF16, tag="vtall")
            wot = pp.tile([128, 2 * D], F32R, tag="wot")
            catq = [pp.tile([128, S], F32R, name=f"catq{h}", tag=f"catq{h}")
                    for h in range(HL)]
            vv = [pp.tile([128, HL * (HD + 1)], F32R, name=f"vv{j}", tag=f"vv{j}")
                  for j in range(JT)]
            xn = [pp.tile([128, S], F32R, name=f"xn{c}", tag=f"xn{c}")
                  for c in range(2)]
            ebias = pp.tile([128, 1], F32, tag="ebias")
            qpk9 = pp.tile([1, 2 * QW + S], F32R, tag="qpk9")
            if has_bo:
                wo9 = pp.tile([1, D], F32R, tag="wo9")
                xn1 = pp.tile([1, 128], F32R, tag="xn1")

            # ---------- input DMAs ----------
            # qpack streamed per 128-row tile across both queues so the
            # qproj accumulation chain can start on the first tile.
            QW2 = 2 * QW + S
            nc.sync.dma_start(qpk9[:], qpack.ap()[D:D + 1, :])
            for c in range(CT):
                eng = nc.sync if c % 2 == 0 else nc.gpsimd
                eng.dma_start(qall[:, c * QW2:(c + 1) * QW2],
                              qpack.ap()[c * 128:(c + 1) * 128, :])
            nc.gpsimd.dma_start(tabs[:], tabs_d.ap())
            nc.gpsimd.dma_start(
                wvall[:].rearrange("p (c n) -> p c n", c=CT),
                wvp_d.ap().rearrange("(c p) n -> p c n", p=128))
            nc.gpsimd.dma_start(
                vtall[:].rearrange("p (c n) -> p c n", c=CT),
                vt_d.ap().rearrange("(c p) n -> p c n", p=128))
            nc.sync.dma_start(
                kgt[:].rearrange("p (h n) -> p h n", h=HL),
                kg_d.ap().rearrange("(h p) n -> p h n", p=128))
            nc.sync.dma_start(
                wot[:].rearrange("p (c n) -> p c n", c=2),
                wo_d.ap()[0:QW, :].rearrange("(c p) n -> p c n", p=128))
            if has_bo:
                nc.sync.dma_start(wo9[:], wo_d.ap()[QW:QW + 1, :])

            # views
            qpk = [qall[:, c * QW2:(c + 1) * QW2] for c in range(CT)]
            kg = [kgt[:, h * S:(h + 1) * S] for h in range(HL)]
            wv = [wvall[:, c * QW:(c + 1) * QW] for c in range(CT)]
            vt = [vtall[:, c * S:(c + 1) * S] for c in range(CT)]
            wo = [wot[:, c * D:(c + 1) * D] for c in range(2)]
            coss = tabs[:, 0:S]
            sinsw = tabs[:, S:2 * S]

            # ---------- small inits ----------
            nc.vector.memset(ebias[:], -25.0)
            if has_bo:
                nc.vector.memset(xn1[:], 1.0)
            for j in range(JT):
                for h in range(HL):
                    nc.vector.memset(
                        vv[j][:, h * (HD + 1) + HD:h * (HD + 1) + HD + 1]
                        .bitcast(F32), 1.0)

            # ---------- phase 1: q projections + rotation ----------
            # qp  = Wq_loc.T @ query.T          (raw q, feeds catq[0:64])
            # qp2 = Wq_sw.T  @ query.T + vb_sw  (head-dim-swapped, feeds the
            #                                    sin term of the rotation)
            with tc.tile_pool(name="qps", bufs=2, space="PSUM") as qps, \
                 tc.tile_pool(name="qp2s", bufs=1, space="PSUM") as qp2s, \
                 tc.tile_pool(name="vps", bufs=2, space="PSUM") as vps:
                for ft in range(2):
                    qp = qps.tile([128, S], F32, tag="qp")
                    qp2 = qp2s.tile([128, S], F32, tag="qp2")
                    for half in range(2):
                        hs = slice(half * 512, half * 512 + 512)
                        qs = slice(2 * QW + half * 512, 2 * QW + half * 512 + 512)
                        for c in range(CT):
                            nc.tensor.matmul(
                                qp[:, hs],
                                qpk[c][:, ft * 128:(ft + 1) * 128],
                                qpk[c][:, qs],
                                start=(c == 0),
                                stop=(c == CT - 1 and not has_bq),
                                skip_group_check=True)
                        if has_bq:
                            nc.tensor.matmul(
                                qp[:, hs], qpk9[:, ft * 128:(ft + 1) * 128],
                                qpk9[:, qs],
                                start=False, stop=True, skip_group_check=True)
                        for c in range(CT):
                            nc.tensor.matmul(
                                qp2[:, hs],
                                qpk[c][:, QW + ft * 128:QW + (ft + 1) * 128],
                                qpk[c][:, qs],
                                start=(c == 0), stop=False,
                                skip_group_check=True)
                        nc.tensor.matmul(
                            qp2[:, hs], qpk9[:, QW + ft * 128:QW + (ft + 1) * 128],
                            qpk9[:, qs],
                            start=False, stop=True, skip_group_check=True)

                    # q rows -> catq[0:64] (ACT partition-shift copies)
                    nc.scalar.copy(catq[2 * ft][0:64, :], qp[0:64, :])
                    nc.scalar.copy(catq[2 * ft + 1][0:64, :], qp[64:128, :])

                    # rotation -> catq rows 64:128:
                    #   t1 = (qp + vb) * cos(w s);  u = qp2 * sinsw
                    #   catq[64:128] = t1 + u   (sin sign folded into sinsw)
                    vbc = tabs[:, 2 * S + ft:2 * S + ft + 1]
                    t1 = sp.tile([128, S], F32, tag="t1")
                    nc.vector.scalar_tensor_tensor(
                        out=t1[:], in0=qp[:], scalar=vbc, in1=coss,
                        op0=ADD, op1=MUL)
                    u_ = sp.tile([128, S], F32, tag="u_")
                    nc.vector.tensor_tensor(out=u_[:], in0=qp2[:], in1=sinsw,
                                            op=MUL)
                    for par in range(2):
                        hq = 2 * ft + par
                        o_ = par * 64
                        nc.vector.tensor_tensor(
                            out=catq[hq][64:128, :], in0=t1[o_:o_ + 64, :],
                            in1=u_[o_:o_ + 64, :], op=ADD)

                # ---------- phase 2: v projection (bf16) ----------
                for jt in range(JT):
                    vp = vps.tile([128, QW], F32, tag="vp")
                    for c in range(CT):
                        nc.tensor.matmul(
                            vp[:], vt[c][:, jt * 128:(jt + 1) * 128], wv[c][:],
                            start=(c == 0), stop=(c == CT - 1),
                            skip_group_check=True)
                    dst = vv[jt][:].rearrange(
                        "p (h x) -> p h x", x=HD + 1)[:, :, 0:HD]
                    src_ = vp[:].rearrange("p (h d) -> p h d", d=HD)
                    nc.scalar.copy(dst, src_)

            # ---------- phase 3: attention ----------
            with tc.tile_pool(name="scps", bufs=2, space="PSUM") as scps, \
                 tc.tile_pool(name="xtps", bufs=2, space="PSUM") as xtps:
                for h in range(HL):
                    xt = xtps.tile([128, S], F32, tag="xt")
                    for jt in range(JT):
                        sc = scps.tile([128, S], F32, tag="sc")
                        for half in range(2):
                            hs = slice(half * 512, half * 512 + 512)
                            nc.tensor.matmul(
                                sc[:, hs],
                                kg[h][:, jt * 128:(jt + 1) * 128],
                                catq[h][:, hs],
                                start=True, stop=True, skip_group_check=True)
                        ex = ep.tile([128, S], F32R, tag="ex")
                        nc.scalar.activation(ex[:], sc[:], EXP,
                                             bias=ebias[:], scale=1.0)
                        for half in range(2):
                            hs = slice(half * 512, half * 512 + 512)
                            nc.tensor.matmul(
                                xt[0:HD + 1, hs],
                                vv[jt][:, h * (HD + 1):(h + 1) * (HD + 1)],
                                ex[:, hs],
                                start=(jt == 0), stop=(jt == JT - 1),
                                skip_group_check=True)
                    # normalize: reciprocal of the denom row, partition-
                    # broadcast it into SBUF, multiply (xt is the only PSUM
                    # operand - verifier allows at most one).
                    dsb = sp.tile([1, S], F32, tag="dsb")
                    nc.vector.tensor_copy(dsb[0:1, :], xt[HD:HD + 1, :])
                    rsb = sp.tile([1, S], F32, tag="rsb")
                    nc.vector.reciprocal_approx_fast(out=rsb[0:1, :],
                                                     in_=dsb[0:1, :])
                    rbs = sp.tile([64, S], F32, tag="rbs")
                    nc.gpsimd.partition_broadcast(rbs[:], rsb[0:1, :])
                    nc.vector.tensor_tensor(
                        out=xn[h // 2][(h % 2) * 64:(h % 2) * 64 + 64, :],
                        in0=xt[0:HD, :], in1=rbs[:], op=MUL)

            # ---------- phase 4: output projection (partial out) ----------
            with tc.tile_pool(name="ops", bufs=2, space="PSUM") as ops, \
                 tc.tile_pool(name="osb", bufs=2) as osb:
                for qt in range(8):
                    op = ops.tile([128, D], F32, tag="op")
                    for half in range(2):
                        hs = slice(half * 512, half * 512 + 512)
                        for c in range(2):
                            nc.tensor.matmul(
                                op[:, hs], xn[c][:, qt * 128:(qt + 1) * 128],
                                wo[c][:, hs],
                                start=(c == 0), stop=(c == 1 and not has_bo),
                                skip_group_check=True)
                        if has_bo:
                            nc.tensor.matmul(
                                op[:, hs], xn1[:], wo9[:, hs],
                                start=False, stop=True, skip_group_check=True)
                    os_ = osb.tile([128, D], F32, tag="os")
                    if qt % 2 == 0:
                        nc.scalar.copy(os_[:], op[:])
                    else:
                        nc.vector.tensor_copy(os_[:], op[:])
                    deng = nc.sync if qt % 2 == 0 else nc.gpsimd
                    deng.dma_start(out_d.ap()[qt * 128:(qt + 1) * 128, :], os_[:])

    nc.finalize()
    return nc


def _host_pack(query, key, value, Wq, bq, Wv, bv, Wo, bo, v_bias):
    """Build the 8 per-core input maps."""
    import ml_dtypes
    r = _rne_fp32r
    bf = ml_dtypes.bfloat16
    QW = HL * HD
    w = np.exp(np.arange(HALF) * (-math.log(10000.0) / (HALF - 1)))

    has_bq = bool(np.any(bq))
    has_bo = bool(np.any(bo)) or bool(np.any(bv))

    # tables shared across the 4 cores of a batch except vb cols
    j = np.arange(S, dtype=np.float64)
    ang_j = w[:, None] * j[None, :]                      # [32, S]
    g64 = np.concatenate([np.sin(ang_j), np.cos(ang_j)], axis=0).astype(np.float32)

    wrep = np.tile(w, 4)[:, None]                        # [128, 1]
    svals = np.arange(S, dtype=np.float64)[None, :]
    cos_ws = np.cos(wrep * svals).astype(np.float32)     # [128, S]
    sin_ws = np.sin(wrep * svals).astype(np.float32)
    # u[p] = qp2[p] * sinsw[p] must give +sin for rows p%64<32 (a_sin) and
    # -sin for rows p%64>=32 (a_cos)
    sinsw = sin_ws.copy()
    sinsw[32:64] *= -1.0
    sinsw[96:128] *= -1.0

    # within-head swap of the 32-dim halves (for the rotation's sin term)
    sw_idx = np.arange(HL * HD)
    sw_idx = (sw_idx // HD) * HD + ((sw_idx % HD) + HALF) % HD

    WqT = Wq.T.astype(np.float32)                        # [D, D]
    WvT = Wv.T.astype(np.float32)
    WoT = Wo.T.astype(np.float32)                        # [Dv, D]

    qTs, kTs, vTs = [], [], []
    for b in range(B):
        qTs.append(np.ascontiguousarray(query[b].T))
        kTs.append(np.ascontiguousarray(key[b].T))
        vTs.append(value[b].T.astype(bf))

    in_maps = []
    for c in range(NC_):
        b, g = c // 4, c % 4
        col0 = g * QW

        WqTl = WqT[:, col0:col0 + QW]
        bql = bq[col0:col0 + QW]
        vbl = v_bias.reshape(-1)[col0:col0 + QW].astype(np.float32)
        qp = np.empty((D + 1, 2 * QW + S), np.float32)
        qp[:D, :QW] = WqTl
        qp[:D, QW:2 * QW] = WqTl[:, sw_idx]
        qp[:D, 2 * QW:] = qTs[b]
        qp[D, :QW] = bql
        qp[D, QW:2 * QW] = (bql + vbl)[sw_idx]
        qp[D, 2 * QW:] = 1.0

        kg = np.empty((HL * 128, S), np.float32)
        for h in range(HL):
            gh = 4 * g + h
            kg[h * 128:h * 128 + 64] = kTs[b][gh * 64:(gh + 1) * 64]
            kg[h * 128 + 64:h * 128 + 128] = g64

        worows = QW + 1 if has_bo else QW
        wop = np.empty((worows, D), np.float32)
        wop[:QW] = WoT[col0:col0 + QW, :]
        if has_bo:
            wop[QW] = bo / 4.0 + bv[col0:col0 + QW] @ WoT[col0:col0 + QW, :]

        tabs = np.empty((128, 2 * S + 2), np.float32)
        tabs[:, 0:S] = cos_ws
        tabs[:, S:2 * S] = sinsw
        for ft in range(2):
            tabs[:, 2 * S + ft] = vbl[ft * 128:(ft + 1) * 128]

        in_maps.append({
            "qpack": r(qp),
            "wvp": WvT[:, col0:col0 + QW].astype(bf),
            "vt": vTs[b],
            "kg": r(kg),
            "wo": r(wop),
            "tabs": tabs,
        })
    return in_maps, has_bq, has_bo


def kernel(query, key, value, mask, Wq, bq, Wv, bv, Wo, bo, v_bias):
    from concourse.bass_utils import run_bass_kernel_spmd

    query = np.asarray(query, np.float32)
    key = np.asarray(key, np.float32)
    value = np.asarray(value, np.float32)
    in_maps, has_bq, has_bo = _host_pack(
        query, key, value,
        np.asarray(Wq, np.float32), np.asarray(bq, np.float32),
        np.asarray(Wv, np.float32), np.asarray(bv, np.float32),
        np.asarray(Wo, np.float32), np.asarray(bo, np.float32),
        np.asarray(v_bias, np.float32))

    ckey = ("nc", has_bq, has_bo)
    if ckey not in _cache:
        _cache[ckey] = _build_nc(has_bq, has_bo)
    nc = _cache[ckey]

    res = run_bass_kernel_spmd(
        nc, in_maps, core_ids=list(range(NC_)),
        trace=bool(int(os.environ.get("BASS_KERNEL_TRACE", "0"))))
    _cache["last_result"] = res

    out = np.empty((B, S, D), np.float32)
    for b in range(B):
        acc = res.results[4 * b]["out"].astype(np.float32)
        for g in range(1, 4):
            acc = acc + res.results[4 * b + g]["out"]
        out[b] = acc
    return out


# revision 31
# speedup vs baseline: 1.6562x; 1.0450x over previous
"""TENER-style MultiHeadedAttention TRN2 kernel (8 NeuronCores, SPMD).

Sharding (tensor-parallel over heads x data-parallel over batch):
core c handles batch b = c//4 and the 4 heads [4*(c%4), 4*(c%4)+4),
over ALL 1024 query rows. Wq/Wv are split column-wise by head, Wo
row-wise; each core emits a PARTIAL output [S, D] and the host gather
sums the 4 partials per batch (the Wo all-reduce).

Key math: the TENER relative-position term after the shift trick is
  rel[s, j] = (q_s + v_bias_h) . pos[j - s]
and pos rows are sinusoids, so by angle addition the whole score is ONE
128-deep contraction per head:
  scores[j, s] = [k_j ; sin(w j) ; cos(w j)] . [q_s ; a_sin(s) ; a_cos(s)]
  a_sin = qv_sin*cos(w s) + qv_cos*sin(w s)
  a_cos = qv_cos*cos(w s) - qv_sin*sin(w s)

All q/k-path matmuls run as float32r (full PE rate at free-dim >= 256).
The v projection runs in bf16 (halves its DMA traffic; v errors are not
exp-amplified). Softmax denominators come free via a ones-column per
head appended to v; normalization broadcasts the denominator row with a
rank-1 PE matmul and divides on DVE.
"""

import math
import os
import sys

sys.path.insert(0, "/opt/trn_rl_repo")

import numpy as np

B, S, D = 2, 1024, 1024
H, HD = 16, 64          # global heads, head_dim
HL = 4                  # local heads per core
HALF = 32               # sin/cos half of head_dim
NC_ = 8                 # cores
JT = S // 128           # 8 key tiles
CT = D // 128           # 8 contraction tiles

_cache: dict = {}


def _rne_fp32r(a):
    """Round fp32 -> fp32r (1s+8e+11m) with round-to-nearest-even."""
    u = np.ascontiguousarray(a, dtype=np.float32).view(np.uint32)
    lsb = (u >> np.uint32(12)) & np.uint32(1)
    return ((u + np.uint32(0x7FF) + lsb) & np.uint32(0xFFFFF000)).view(np.float32)


def _build_nc(has_bq: bool, has_bo: bool):
    import concourse.bacc as bacc
    import concourse.mybir as mybir
    from concourse import tile

    F32 = mybir.dt.float32
    F32R = mybir.dt.float32r
    BF16 = mybir.dt.bfloat16
    ADD = mybir.AluOpType.add
    MUL = mybir.AluOpType.mult
    DIV = mybir.AluOpType.divide
    EXP = mybir.ActivationFunctionType.Exp

    nc = bacc.Bacc("TRN2", target_bir_lowering=False, debug=False, num_devices=NC_)

    QW = HL * HD                      # 256 local q / v feature cols
    worows = QW + 1 if has_bo else QW
    # qpack: [Wq_loc.T (256) | query.T (S)] plus a bias row [bq_loc | ones]
    qpack = nc.dram_tensor("qpack", [D + 1, QW + S], F32R,
                           kind="ExternalInput")
    wvp_d = nc.dram_tensor("wvp", [D, QW], BF16, kind="ExternalInput")
    vt_d = nc.dram_tensor("vt", [D, S], BF16, kind="ExternalInput")
    kg_d = nc.dram_tensor("kg", [HL * 128, S], F32R, kind="ExternalInput")
    wo_d = nc.dram_tensor("wo", [worows, D], F32R, kind="ExternalInput")
    # tabs: [128, 1024 coss | 1024 sinsw | 2 vb-cols]
    tabs_d = nc.dram_tensor("tabs", [128, 2 * S + 2], F32, kind="ExternalInput")
    out_d = nc.dram_tensor("out", [S, D], F32, kind="ExternalOutput")

    with tile.TileContext(nc, num_cores=NC_) as tc:
        with tc.tile_pool(name="persist", bufs=1) as pp, \
             tc.tile_pool(name="scratch", bufs=2) as sp, \
             tc.tile_pool(name="exppool", bufs=3) as ep:

            # ---------- persistent SBUF ----------
            tabs = pp.tile([128, 2 * S + 2], F32, tag="tabs")
            qall = pp.tile([128, CT * (QW + S)], F32R, tag="qall")
            kgt = pp.tile([128, HL * S], F32R, tag="kgt")
            wvall = pp.tile([128, CT * QW], BF16, tag="wvall")
            vtall = pp.tile([128, CT * S], BF16, tag="vtall")
            wot = pp.tile([128, 2 * D], F32R, tag="wot")
            catq = [pp.tile([128, S], F32R, name=f"catq{h}", tag=f"catq{h}")
                    for h in range(HL)]
            vv = [pp.tile([128, HL * (HD + 1)], F32R, name=f"vv{j}", tag=f"vv{j}")
                  for j in range(JT)]
            xn = [pp.tile([128, S], F32R, name=f"xn{c}", tag=f"xn{c}")
                  for c in range(2)]
            ebias = pp.tile([128, 1], F32, tag="ebias")
            if has_bq:
                qpk9 = pp.tile([1, QW + S], F32R, tag="qpk9")
            if has_bo:
                wo9 = pp.tile([1, D], F32R, tag="wo9")
                xn1 = pp.tile([1, 128], F32R, tag="xn1")

            # ---------- input DMAs ----------
            # qpack streamed per 128-row tile across both queues so the
            # qproj accumulation chain can start on the first tile.
            QW2 = QW + S
            if has_bq:
                nc.sync.dma_start(qpk9[:], qpack.ap()[D:D + 1, :])
            for c in range(CT):
                eng = nc.sync if c % 2 == 0 else nc.gpsimd
                eng.dma_start(qall[:, c * QW2:(c + 1) * QW2],
                              qpack.ap()[c * 128:(c + 1) * 128, :])
            nc.sync.dma_start(tabs[:], tabs_d.ap())
            nc.gpsimd.dma_start(
                wvall[:].rearrange("p (c n) -> p c n", c=CT),
                wvp_d.ap().rearrange("(c p) n -> p c n", p=128))
            nc.gpsimd.dma_start(
                vtall[:].rearrange("p (c n) -> p c n", c=CT),
                vt_d.ap().rearrange("(c p) n -> p c n", p=128))
            for h in range(HL):
                nc.sync.dma_start(
                    kgt[:, h * S:(h + 1) * S],
                    kg_d.ap()[h * 128:(h + 1) * 128, :])
            nc.sync.dma_start(
                wot[:].rearrange("p (c n) -> p c n", c=2),
                wo_d.ap()[0:QW, :].rearrange("(c p) n -> p c n", p=128))
            if has_bo:
                nc.sync.dma_start(wo9[:], wo_d.ap()[QW:QW + 1, :])

            # views
            qpk = [qall[:, c * QW2:(c + 1) * QW2] for c in range(CT)]
            kg = [kgt[:, h * S:(h + 1) * S] for h in range(HL)]
            wv = [wvall[:, c * QW:(c + 1) * QW] for c in range(CT)]
            vt = [vtall[:, c * S:(c + 1) * S] for c in range(CT)]
            wo = [wot[:, c * D:(c + 1) * D] for c in range(2)]
            coss = tabs[:, 0:S]
            sinsw = tabs[:, S:2 * S]

            # ---------- small inits ----------
            nc.vector.memset(ebias[:], -25.0)
            if has_bo:
                nc.vector.memset(xn1[:], 1.0)
            for j in range(JT):
                for h in range(HL):
                    nc.vector.memset(
                        vv[j][:, h * (HD + 1) + HD:h * (HD + 1) + HD + 1]
                        .bitcast(F32), 1.0)

            # ---------- phase 1: q projection + rotation ----------
            with tc.tile_pool(name="qps", bufs=2, space="PSUM") as qps, \
                 tc.tile_pool(name="vps", bufs=2, space="PSUM") as vps:
                for ft in range(2):
                    qp = qps.tile([128, S], F32, tag="qp")
                    for half in range(2):
                        hs = slice(half * 512, half * 512 + 512)
                        qs = slice(QW + half * 512, QW + half * 512 + 512)
                        for c in range(CT):
                            nc.tensor.matmul(
                                qp[:, hs],
                                qpk[c][:, ft * 128:(ft + 1) * 128],
                                qpk[c][:, qs],
                                start=(c == 0),
                                stop=(c == CT - 1 and not has_bq),
                                skip_group_check=True)
                        if has_bq:
                            nc.tensor.matmul(
                                qp[:, hs], qpk9[:, ft * 128:(ft + 1) * 128],
                                qpk9[:, qs],
                                start=False, stop=True, skip_group_check=True)

                    # q rows -> catq[0:64] (ACT partition-shift copies)
                    nc.scalar.copy(catq[2 * ft][0:64, :], qp[0:64, :])
                    nc.scalar.copy(catq[2 * ft + 1][0:64, :], qp[64:128, :])

                    # rotation -> catq rows 64:128
                    vbc = tabs[:, 2 * S + ft:2 * S + ft + 1]
                    t1 = sp.tile([128, S], F32, tag="t1")
                    nc.vector.scalar_tensor_tensor(
                        out=t1[:], in0=qp[:], scalar=vbc, in1=coss,
                        op0=ADD, op1=MUL)
                    # v2s[p] = (qp[swap32(p)] + vb[swap32(p)]) * sinsw[swap32(p)]
                    # (32-block swap folded into the shifted output base; the
                    # sign of the sin factor is folded into the sinsw table)
                    v2s = sp.tile([128, S], F32, tag="v2s")
                    for blk in range(4):
                        si = 32 * (blk ^ 1)
                        nc.vector.scalar_tensor_tensor(
                            out=v2s[32 * blk:32 * blk + 32, :],
                            in0=qp[si:si + 32, :],
                            scalar=tabs[si:si + 32, 2 * S + ft:2 * S + ft + 1],
                            in1=sinsw[si:si + 32, :],
                            op0=ADD, op1=MUL)
                    for par in range(2):
                        hq = 2 * ft + par
                        o_ = par * 64
                        nc.vector.tensor_tensor(
                            out=catq[hq][64:128, :], in0=t1[o_:o_ + 64, :],
                            in1=v2s[o_:o_ + 64, :], op=ADD)

                # ---------- phase 2: v projection (bf16) ----------
                for jt in range(JT):
                    vp = vps.tile([128, QW], F32, tag="vp")
                    for c in range(CT):
                        nc.tensor.matmul(
                            vp[:], vt[c][:, jt * 128:(jt + 1) * 128], wv[c][:],
                            start=(c == 0), stop=(c == CT - 1),
                            skip_group_check=True)
                    dst = vv[jt][:].rearrange(
                        "p (h x) -> p h x", x=HD + 1)[:, :, 0:HD]
                    src_ = vp[:].rearrange("p (h d) -> p h d", d=HD)
                    nc.scalar.copy(dst, src_)

            # ---------- phase 3: attention ----------
            with tc.tile_pool(name="scps", bufs=2, space="PSUM") as scps, \
                 tc.tile_pool(name="xtps", bufs=2, space="PSUM") as xtps:
                for h in range(HL):
                    xt = xtps.tile([128, S], F32, tag="xt")
                    for jt in range(JT):
                        sc = scps.tile([128, S], F32, tag="sc")
                        for half in range(2):
                            hs = slice(half * 512, half * 512 + 512)
                            nc.tensor.matmul(
                                sc[:, hs],
                                kg[h][:, jt * 128:(jt + 1) * 128],
                                catq[h][:, hs],
                                start=True, stop=True, skip_group_check=True)
                        ex = ep.tile([128, S], F32R, tag="ex")
                        nc.scalar.activation(ex[:], sc[:], EXP,
                                             bias=ebias[:], scale=1.0)
                        for half in range(2):
                            hs = slice(half * 512, half * 512 + 512)
                            nc.tensor.matmul(
                                xt[0:HD + 1, hs],
                                vv[jt][:, h * (HD + 1):(h + 1) * (HD + 1)],
                                ex[:, hs],
                                start=(jt == 0), stop=(jt == JT - 1),
                                skip_group_check=True)
                    # normalize (per q-half so the last half overlaps the
                    # next phase): reciprocal of the denom row, partition-
                    # broadcast it into SBUF, multiply (xt is the only PSUM
                    # operand - verifier allows at most one).
                    dsb = sp.tile([1, S], F32, tag="dsb")
                    rsb = sp.tile([1, S], F32, tag="rsb")
                    rbs = sp.tile([64, S], F32, tag="rbs")
                    for half in range(2):
                        hs = slice(half * 512, half * 512 + 512)
                        nc.vector.tensor_copy(dsb[0:1, hs], xt[HD:HD + 1, hs])
                        nc.vector.reciprocal_approx_fast(out=rsb[0:1, hs],
                                                         in_=dsb[0:1, hs])
                        nc.gpsimd.partition_broadcast(rbs[:, hs], rsb[0:1, hs])
                        nc.vector.tensor_tensor(
                            out=xn[h // 2][(h % 2) * 64:(h % 2) * 64 + 64, hs],
                            in0=xt[0:HD, hs], in1=rbs[:, hs], op=MUL)

            # ---------- phase 4: output projection (partial out) ----------
            with tc.tile_pool(name="ops", bufs=2, space="PSUM") as ops, \
                 tc.tile_pool(name="osb", bufs=2) as osb:
                for qt in range(8):
                    op = ops.tile([128, D], F32, tag="op")
                    for half in range(2):
                        hs = slice(half * 512, half * 512 + 512)
                        for c in range(2):
                            nc.tensor.matmul(
                                op[:, hs], xn[c][:, qt * 128:(qt + 1) * 128],
                                wo[c][:, hs],
                                start=(c == 0), stop=(c == 1 and not has_bo),
                                skip_group_check=True)
                        if has_bo:
                            nc.tensor.matmul(
                                op[:, hs], xn1[:], wo9[:, hs],
                                start=False, stop=True, skip_group_check=True)
                    os_ = osb.tile([128, D], F32, tag="os")
                    if qt % 2 == 0:
                        nc.scalar.copy(os_[:], op[:])
                    else:
                        nc.vector.tensor_copy(os_[:], op[:])
                    deng = nc.sync if qt % 2 == 0 else nc.gpsimd
                    deng.dma_start(out_d.ap()[qt * 128:(qt + 1) * 128, :], os_[:])

    nc.finalize()
    return nc


def _host_pack(query, key, value, Wq, bq, Wv, bv, Wo, bo, v_bias):
    """Build the 8 per-core input maps."""
    import ml_dtypes
    r = _rne_fp32r
    bf = ml_dtypes.bfloat16
    QW = HL * HD
    w = np.exp(np.arange(HALF) * (-math.log(10000.0) / (HALF - 1)))

    has_bq = bool(np.any(bq))
    has_bo = bool(np.any(bo)) or bool(np.any(bv))

    # tables shared across the 4 cores of a batch except vb cols
    j = np.arange(S, dtype=np.float64)
    ang_j = w[:, None] * j[None, :]                      # [32, S]
    g64 = np.concatenate([np.sin(ang_j), np.cos(ang_j)], axis=0).astype(np.float32)

    wrep = np.tile(w, 4)[:, None]                        # [128, 1]
    svals = np.arange(S, dtype=np.float64)[None, :]
    cos_ws = np.cos(wrep * svals).astype(np.float32)     # [128, S]
    sin_ws = np.sin(wrep * svals).astype(np.float32)
    # v2s[p] = qv[swap(p)] * sinsw[swap(p)]: the sin factor needs sign -1 on
    # the rows feeding a_cos, i.e. sinsw rows with p%64 < 32 are negated
    sinsw = sin_ws.copy()
    sinsw[0:32] *= -1.0
    sinsw[64:96] *= -1.0

    WqT = Wq.T.astype(np.float32)                        # [D, D]
    WvT = Wv.T.astype(np.float32)
    WoT = Wo.T.astype(np.float32)                        # [Dv, D]

    qTs, kTs, vTs = [], [], []
    for b in range(B):
        qTs.append(np.ascontiguousarray(query[b].T))
        kTs.append(np.ascontiguousarray(key[b].T))
        vTs.append(value[b].T.astype(bf))

    in_maps = []
    for c in range(NC_):
        b, g = c // 4, c % 4
        col0 = g * QW

        WqTl = WqT[:, col0:col0 + QW]
        bql = bq[col0:col0 + QW]
        vbl = v_bias.reshape(-1)[col0:col0 + QW].astype(np.float32)
        qp = np.empty((D + 1, QW + S), np.float32)
        qp[:D, :QW] = WqTl
        qp[:D, QW:] = qTs[b]
        qp[D, :QW] = bql
        qp[D, QW:] = 1.0

        kg = np.empty((HL * 128, S), np.float32)
        for h in range(HL):
            gh = 4 * g + h
            kg[h * 128:h * 128 + 64] = kTs[b][gh * 64:(gh + 1) * 64]
            kg[h * 128 + 64:h * 128 + 128] = g64

        worows = QW + 1 if has_bo else QW
        wop = np.empty((worows, D), np.float32)
        wop[:QW] = WoT[col0:col0 + QW, :]
        if has_bo:
            wop[QW] = bo / 4.0 + bv[col0:col0 + QW] @ WoT[col0:col0 + QW, :]

        tabs = np.empty((128, 2 * S + 2), np.float32)
        tabs[:, 0:S] = cos_ws
        tabs[:, S:2 * S] = sinsw
        for ft in range(2):
            tabs[:, 2 * S + ft] = vbl[ft * 128:(ft + 1) * 128]

        in_maps.append({
            "qpack": r(qp),
            "wvp": WvT[:, col0:col0 + QW].astype(bf),
            "vt": vTs[b],
            "kg": r(kg),
            "wo": r(wop),
            "tabs": tabs,
        })
    return in_maps, has_bq, has_bo


def kernel(query, key, value, mask, Wq, bq, Wv, bv, Wo, bo, v_bias):
    from concourse.bass_utils import run_bass_kernel_spmd

    query = np.asarray(query, np.float32)
    key = np.asarray(key, np.float32)
    value = np.asarray(value, np.float32)
    in_maps, has_bq, has_bo = _host_pack(
        query, key, value,
        np.asarray(Wq, np.float32), np.asarray(bq, np.float32),
        np.asarray(Wv, np.float32), np.asarray(bv, np.float32),
        np.asarray(Wo, np.float32), np.asarray(bo, np.float32),
        np.asarray(v_bias, np.float32))

    ckey = ("nc", has_bq, has_bo)
    if ckey not in _cache:
        _cache[ckey] = _build_nc(has_bq, has_bo)
    nc = _cache[ckey]

    res = run_bass_kernel_spmd(
        nc, in_maps, core_ids=list(range(NC_)),
        trace=bool(int(os.environ.get("BASS_KERNEL_TRACE", "0"))))
    _cache["last_result"] = res

    out = np.empty((B, S, D), np.float32)
    for b in range(B):
        acc = res.results[4 * b]["out"].astype(np.float32)
        for g in range(1, 4):
            acc = acc + res.results[4 * b + g]["out"]
        out[b] = acc
    return out
